# revision 17
# baseline (speedup 1.0000x reference)
"""Trainium2 Bass kernel for nn_DiscretePatternLayer.

Strategy (8 NeuronCores, data-parallel over batch B=8, one batch row per core):

  * Everything on-device is kept feature-major ([D, tokens]) so the matmul
    chain needs no transposes; the host transposes inputs/outputs (numpy).
  * The "paradox" d = h1@W2 + b2 - h1 is folded into host-precomputed
    weights:  d = x @ (W1 @ (W2 - I)) + const-bias  (one complex matmul
    straight from x, accumulated in PSUM).
  * Pattern attention never materializes comp vectors.  pe = mean(
    (next_comp-my_comp)^2 ) is a quadratic form in the two softmax weight
    vectors with a host-precomputed 16x16 Gram matrix of the pattern banks.
  * The two global scalar reductions (mean(pe), routing cost) are handled
    with a single AllGather of the per-token pe vector (16 KB per core);
    every core redundantly computes the global stats (cheap).
  * Token chunks of 512 (one PSUM bank per [128,512] fp32 tile).
"""

import functools

import numpy as np

import concourse.bacc as bacc
import concourse.mybir as mybir
import concourse.tile as tile
from concourse.bass import ds

# ----------------------------------------------------------------------
# All ACT functions this kernel uses ({ln, exp, square, abs, copy}) live
# together in the `natural_log_exp_and_others` table set, but bacc's
# load-insertion pass greedily assigns each function to the FIRST set
# containing it (ln -> natural_log, exp -> exp_and_others), forcing a
# ~2.7us table reload between every ln/exp pair.  Patch the table map so
# functions of the shared set resolve only to it (set ids preserved).
# ----------------------------------------------------------------------
import concourse.hw_specs as _hw_specs  # noqa: E402

_SHARED_SET = "natural_log_exp_and_others"
if not getattr(_hw_specs.get_activation_tables, "_dpl_patched", False):
    _orig_get_tables = _hw_specs.get_activation_tables

    @functools.cache
    def _patched_get_tables(module_arch):
        t = _orig_get_tables(module_arch)
        if _SHARED_SET not in t:
            return t
        shared = t[_SHARED_SET]
        return {name: (fns if name == _SHARED_SET else fns - shared)
                for name, fns in t.items()}

    _patched_get_tables._dpl_patched = True
    _hw_specs.get_activation_tables = _patched_get_tables
    bacc.get_activation_tables = _patched_get_tables

F32 = mybir.dt.float32
FR = mybir.dt.float32r   # "rounded" fp32: full-rate PE (1cyc/row), ~13-bit mantissa
AF = mybir.ActivationFunctionType
ALU = mybir.AluOpType

N_CORES = 8
B, L, D, P = 8, 4096, 256, 8
CH = 512            # tokens per chunk
NCH = L // CH       # chunks per core
KT = D // 128       # k-tiles per feature dim (2)
NTOK = B * L        # global token count

WNAMES = ["w1r", "w1i", "w1in",      # h1 = x @ W1
          "wdr", "wdi", "wdin",      # d  = x @ (W1 @ (W2 - I))
          "nw1r", "nw1i", "nw1in",   # a1 = h @ nW1
          "nwdr", "nwdi", "nwdin",   # da = h @ (nW1 @ (nW2 - I))
          "w3r", "w3i", "w3in"]      # pf = h @ W3


class _Pair:
    """Adapter so two separate [128, CH] tiles index like [128, KT, CH]."""

    def __init__(self, t0, t1):
        self.t = (t0, t1)

    def __getitem__(self, idx):
        _, k, sl = idx
        return self.t[k][:, sl]


def _build_nc(with_bias: bool, single_core: bool = False,
              use_gpsimd: bool = True, use_collective: bool = True):
    ncore = 1 if single_core else N_CORES
    nc = bacc.Bacc("TRN2", target_bir_lowering=False, debug=False,
                   num_devices=ncore)

    dt = lambda name, shape, dty=F32: nc.dram_tensor(
        name, shape, dty, kind="ExternalInput").ap()
    do = lambda name, shape, dty=F32: nc.dram_tensor(
        name, shape, dty, kind="ExternalOutput").ap()

    x_r = dt("x_r", [D, L], FR)
    x_i = dt("x_i", [D, L], FR)
    W = {n: dt(n, [D, D], FR) for n in WNAMES}
    pats = {n: dt(n, [D, P], FR) for n in ["spmr", "spmi", "spnr", "spni"]}
    gr = {n: dt(n, [P, P], FR) for n in ["gtm", "gtn", "gbm", "gbn"]}
    sels = dt("sels", [P, P * NCH], FR)      # pe row-selector, col block c
    bcsel = dt("bcsel", [P, 128 * NCH], FR)  # conf broadcast selector
    on8 = dt("on8", [P, 1], FR)
    on8f = dt("on8f", [P, 1])
    on64 = dt("on64", [N_CORES * P, 1])
    o1x64 = dt("o1x64", [1, N_CORES * P], FR)
    o1x64f = dt("o1x64f", [1, N_CORES * P])
    bias = {}
    if with_bias:
        bias = {n: dt(n, [2, D], FR)
                for n in ["b1", "bd", "nb1", "nbd", "b3"]}
        onesrow = dt("onesrow", [1, CH], FR)

    cu_r = do("cu_r", [D, L], FR)
    cu_i = do("cu_i", [D, L], FR)
    pc_r = do("pc_r", [D, L])
    pc_i = do("pc_i", [D, L])
    pe8 = do("pe8", [NCH, CH])

    r3 = lambda ap: ap.rearrange("(kt k) t -> k kt t", k=128)

    with tile.TileContext(nc) as tc:
        with (
            tc.tile_pool(name="consts", bufs=1) as consts,
            tc.tile_pool(name="hbuf", bufs=1) as hbuf,
            tc.tile_pool(name="xin", bufs=2) as xin,
            tc.tile_pool(name="work", bufs=2) as work,
            tc.tile_pool(name="small", bufs=2) as small,
            tc.tile_pool(name="stage", bufs=1) as stage,
            tc.tile_pool(name="outb", bufs=2) as outb,
            tc.tile_pool(name="psD", bufs=2, space="PSUM") as psD,
            tc.tile_pool(name="psH1", bufs=2, space="PSUM") as psH1,
            tc.tile_pool(name="psS", bufs=2, space="PSUM") as psS,
            tc.tile_pool(name="dram", bufs=1, space="DRAM") as dram,
        ):
            # ---------------- constants into SBUF ----------------
            # load the weights needed by the first matmuls first so the PE
            # can start while the rest stream in
            first_w = ["wdr", "wdin", "wdi", "w1r", "w1in", "w1i"]
            wt = {}
            for n in first_w + [n for n in WNAMES if n not in first_w]:
                wt[n] = consts.tile([128, KT, D], FR, name=f"wt_{n}")
                nc.sync.dma_start(wt[n][:], r3(W[n]))
            pt = {}
            for n in pats:
                pt[n] = consts.tile([128, KT, P], FR, name=f"pt_{n}")
                nc.sync.dma_start(pt[n][:], r3(pats[n]))
            gt = {}
            for n in gr:
                gt[n] = consts.tile([P, P], FR, name=f"gt_{n}")
                nc.sync.dma_start(gt[n][:], gr[n][:])
            sel_t = consts.tile([P, P * NCH], FR)
            nc.sync.dma_start(sel_t[:], sels[:])
            bc_t = consts.tile([P, 128 * NCH], FR)
            nc.sync.dma_start(bc_t[:], bcsel[:])
            on8_t = consts.tile([P, 1], FR)
            nc.sync.dma_start(on8_t[:], on8[:])
            on8f_t = consts.tile([P, 1], F32)
            nc.sync.dma_start(on8f_t[:], on8f[:])
            on64_t = consts.tile([N_CORES * P, 1], F32)
            nc.sync.dma_start(on64_t[:], on64[:])
            o1x64_t = consts.tile([1, N_CORES * P], FR)
            nc.sync.dma_start(o1x64_t[:], o1x64[:])
            o1x64f_t = consts.tile([1, N_CORES * P], F32)
            nc.sync.dma_start(o1x64f_t[:], o1x64f[:])
            bias_t = {}
            ones_row = None
            if with_bias:
                for n in bias:
                    bias_t[n] = consts.tile([2, D], FR, name=f"bias_{n}")
                    nc.sync.dma_start(bias_t[n][:], bias[n][:])
                ones_row = consts.tile([1, CH], FR)
                nc.sync.dma_start(ones_row[:], onesrow[:])

            # persistent h (feature-major, full core-local sequence)
            h_r = hbuf.tile([128, KT, L], FR)
            h_i = hbuf.tile([128, KT, L], FR)

            pe0_sb = stage.tile([P, CH], F32)  # accumulated across chunks
            nc.vector.memset(pe0_sb[:], 0.0)

            def lin_mm(psum, wa, wb, rhs_r, rhs_i, f, bias_name, comp):
                """psum[128,CH] = rhs_r @ Wa + rhs_i @ Wb (+bias), out tile f."""
                ms = ds(128 * f, 128)
                nb = with_bias and bias_name is not None
                for j, (w_, rhs) in enumerate([(wa, rhs_r), (wb, rhs_i)]):
                    for k in range(KT):
                        nc.tensor.matmul(
                            psum[:], wt[w_][:, k, ms], rhs[:, k, slice(None)],
                            start=(j == 0 and k == 0),
                            stop=(j == 1 and k == KT - 1 and not nb))
                if nb:
                    nc.tensor.matmul(
                        psum[:], bias_t[bias_name][comp:comp + 1, ms],
                        ones_row[:], start=False, stop=True)

            def paradox_gate(xr_, xi_, wpre, bias_d, bias_1, tag, outs):
                """outs: 2 APs [128, KT, CH] receiving gated h (r, i)."""
                sqr = work.tile([128, KT, CH], F32, tag="sqr", bufs=1,
                                name=f"sqr_{tag}")
                g2 = work.tile([128, KT, CH], F32, tag="g", name=f"g_{tag}")
                for f in range(KT):
                    d_r = psD.tile([128, CH], F32, tag="psD",
                                   name=f"dr_{tag}{f}")
                    d_i = psD.tile([128, CH], F32, tag="psD",
                                   name=f"di_{tag}{f}")
                    lin_mm(d_r, wpre + "wdr", wpre + "wdin",
                           xr_, xi_, f, bias_d, 0)
                    lin_mm(d_i, wpre + "wdi", wpre + "wdr",
                           xr_, xi_, f, bias_d, 1)
                    nc.scalar.activation(sqr[:, f, :], d_r[:], AF.Square)
                    nc.scalar.activation(g2[:, f, :], d_i[:], AF.Square)
                # g = sigmoid(sqrt(m)) via {ln, exp} (single ACT table set),
                # batched over both feature tiles
                eng_add = nc.gpsimd if use_gpsimd else nc.vector
                eng_add.tensor_add(g2[:], g2[:], sqr[:])
                nc.scalar.activation(g2[:], g2[:], AF.Ln)
                nc.scalar.activation(g2[:], g2[:], AF.Exp, scale=0.5)
                nc.scalar.activation(g2[:], g2[:], AF.Exp, scale=-1.0)
                nc.vector.tensor_scalar_add(g2[:], g2[:], 1.0)
                nc.vector.reciprocal(g2[:], g2[:])
                for ci, comp in enumerate(("r", "i")):
                    if comp == "r":
                        wa, wb = wpre + "w1r", wpre + "w1in"
                    else:
                        wa, wb = wpre + "w1i", wpre + "w1r"
                    h1p = psH1.tile([128, KT, CH], F32, tag="psH1",
                                    name=f"h1_{tag}_{comp}")
                    for f in range(KT):
                        lin_mm(h1p[:, f, :], wa, wb, xr_, xi_, f, bias_1, ci)
                    nc.vector.tensor_mul(outs[ci], h1p[:], g2[:])

            def attn_w(t_r, t_i, pre, tag):
                """softmax weights [8, CH] (sbuf) for pattern bank `pre`."""
                sc = psS.tile([P, CH], F32, tag="ps_small", name=f"sc_{tag}")
                tls = {"r": t_r, "i": t_i}
                for j, comp in enumerate(("r", "i")):
                    for k in range(KT):
                        nc.tensor.matmul(
                            sc[:], pt["sp" + pre + comp][:, k, :],
                            tls[comp][:, k, :],
                            start=(j == 0 and k == 0),
                            stop=(j == 1 and k == KT - 1))
                e = small.tile([P, CH], FR, tag="e", name=f"e_{tag}")
                nc.scalar.activation(e[:], sc[:], AF.Exp)
                z = psS.tile([1, CH], F32, tag="ps_small", name=f"z_{tag}")
                nc.tensor.matmul(z[:], on8_t[:], e[:], start=True, stop=True)
                rz = small.tile([1, CH], FR, tag="rz", name=f"rz_{tag}")
                with nc.allow_low_precision(reason="f32r softmax denom"):
                    nc.vector.reciprocal(rz[:], z[:])
                zb = psS.tile([P, CH], F32, tag="ps_small", name=f"zb_{tag}")
                nc.tensor.matmul(zb[:], o1x64_t[:, :P], rz[:],
                                 start=True, stop=True)
                w_ = small.tile([P, CH], FR, tag="w", bufs=4,
                                name=f"w_{tag}")
                nc.vector.tensor_mul(w_[:], e[:], zb[:])
                return w_

            # ====================== phase 1 ======================
            for c in range(NCH):
                csl = ds(CH * c, CH)
                xr = xin.tile([128, KT, CH], FR, tag="xr", name=f"xr_{c}")
                nc.sync.dma_start(xr[:], r3(x_r)[:, :, csl])
                xi = xin.tile([128, KT, CH], FR, tag="xi", name=f"xi_{c}")
                nc.sync.dma_start(xi[:], r3(x_i)[:, :, csl])

                hr_c = h_r[:, :, csl]
                hi_c = h_i[:, :, csl]
                paradox_gate(xr, xi, "", "bd", "b1", f"p{c}",
                             [hr_c, hi_c])

                a_r = work.tile([128, KT, CH], FR, tag="ar", name=f"ar_{c}")
                a_i = work.tile([128, KT, CH], FR, tag="ai", name=f"ai_{c}")
                paradox_gate(hr_c, hi_c, "n", "nbd", "nb1", f"n{c}",
                             [a_r[:], a_i[:]])

                w_my = attn_w(hr_c, hi_c, "m", f"m{c}")
                w_nx = attn_w(a_r[:], a_i[:], "n", f"n{c}")

                u_top = psS.tile([P, CH], F32, tag="ps_small", name=f"ut_{c}")
                nc.tensor.matmul(u_top[:], gt["gtm"][:], w_my[:],
                                 start=True, stop=False)
                nc.tensor.matmul(u_top[:], gt["gtn"][:], w_nx[:],
                                 start=False, stop=True)
                u_bot = psS.tile([P, CH], F32, tag="ps_small", name=f"ub_{c}")
                nc.tensor.matmul(u_bot[:], gt["gbm"][:], w_my[:],
                                 start=True, stop=False)
                nc.tensor.matmul(u_bot[:], gt["gbn"][:], w_nx[:],
                                 start=False, stop=True)
                pm = small.tile([P, CH], FR, tag="pm", name=f"pm_{c}")
                nc.vector.tensor_mul(pm[:], w_my[:], u_top[:])
                pn_ = small.tile([P, CH], FR, tag="pn", name=f"pn_{c}")
                nc.vector.tensor_mul(pn_[:], w_nx[:], u_bot[:])
                ssl = ds(P * c, P)
                pe_c = psS.tile([P, CH], F32, tag="ps_small",
                                name=f"pec_{c}")
                nc.tensor.matmul(pe_c[:], sel_t[:, ssl], pm[:],
                                 start=True, stop=False)
                nc.tensor.matmul(pe_c[:], sel_t[:, ssl], pn_[:],
                                 start=False, stop=True)
                nc.vector.tensor_add(pe0_sb[:], pe0_sb[:], pe_c[:])

            # ====================== phase 2 ======================
            pe0_s = pe0_sb
            s8 = stage.tile([P, 1], F32)
            nc.vector.reduce_sum(s8[:], pe0_s[:], axis=mybir.AxisListType.X)

            bounce_in = dram.tile([P, CH], F32)
            nc.sync.dma_start(bounce_in[:], pe0_s[:])
            bounce_out = dram.tile([N_CORES * P, CH], F32)
            if single_core or not use_collective:
                # stand-in with similar traffic for TimelineSim analysis
                for cc in range(N_CORES):
                    nc.sync.dma_start(bounce_out[ds(P * cc, P), :],
                                      bounce_in[:])
            else:
                nc.gpsimd.collective_compute(
                    "AllGather", ALU.bypass,
                    replica_groups=[list(range(N_CORES))],
                    ins=[bounce_in.opt()], outs=[bounce_out.opt()],
                )
            pall = stage.tile([N_CORES * P, CH], F32)
            nc.sync.dma_start(pall[:], bounce_out[:])

            tot_ps = psS.tile([1, 1], F32, tag="ps_small")
            nc.tensor.matmul(tot_ps[:], on8f_t[:], s8[:], start=True,
                             stop=True)
            t_s = stage.tile([1, 1], F32)
            nc.scalar.activation(t_s[:], tot_ps[:], AF.Copy, scale=1.0 / NTOK)
            m64_ps = psS.tile([N_CORES * P, 1], F32, tag="ps_small")
            nc.tensor.matmul(m64_ps[:], o1x64f_t[:], t_s[:],
                             start=True, stop=True)
            m64n = stage.tile([N_CORES * P, 1], F32)
            nc.scalar.activation(m64n[:], m64_ps[:], AF.Copy, scale=-1.0)

            def sigmoid_of(dst, src):
                # dst = 1 / (1 + exp(-src))
                nc.scalar.activation(dst, src, AF.Exp, scale=-1.0)
                nc.vector.tensor_scalar_add(dst, dst, 1.0)
                nc.vector.reciprocal(dst, dst)

            def tanh_neg_of(dst, src, pool_tag, nparts):
                # dst = tanh(-src) = (q - 1) / (q + 1), q = exp(-2 src)
                q = stage.tile([nparts, CH], F32, tag=pool_tag,
                               bufs=3 if pool_tag == "st64" else 4,
                               name=f"q_{pool_tag}")
                nc.scalar.activation(q[:], src, AF.Exp, scale=-2.0)
                den = stage.tile([nparts, CH], F32, tag=pool_tag,
                                 bufs=3 if pool_tag == "st64" else 4,
                                 name=f"den_{pool_tag}")
                nc.vector.tensor_scalar_add(den[:], q[:], 1.0)
                nc.vector.reciprocal(den[:], den[:])
                nc.vector.tensor_scalar_add(q[:], q[:], -1.0)
                nc.vector.tensor_mul(dst, q[:], den[:])

            cert = stage.tile([N_CORES * P, CH], F32, tag="st64", bufs=3)
            nc.scalar.activation(cert[:], pall[:], AF.Abs, bias=m64n[:])
            temp = stage.tile([N_CORES * P, CH], F32, tag="st64", bufs=3)
            sigmoid_of(temp[:], cert[:])
            pt64 = stage.tile([N_CORES * P, CH], F32, tag="st64", bufs=3)
            nc.vector.tensor_mul(pt64[:], pall[:], temp[:])
            th64 = stage.tile([N_CORES * P, CH], F32, tag="st64", bufs=3)
            tanh_neg_of(th64[:], pt64[:], "st64", N_CORES * P)
            sq64 = stage.tile([N_CORES * P, CH], F32, tag="st64", bufs=3)
            ss64 = stage.tile([N_CORES * P, 1], F32)
            nc.scalar.activation(sq64[:], th64[:], AF.Square,
                                 accum_out=ss64[:])
            tot2_ps = psS.tile([1, 1], F32, tag="ps_small")
            nc.tensor.matmul(tot2_ps[:], on64_t[:], ss64[:],
                             start=True, stop=True)
            rc_s = stage.tile([1, 1], F32)
            # rc = 0.1 * mean(conf*(1-conf)) = 0.025*(1 - mean(th^2))
            nc.scalar.activation(rc_s[:], tot2_ps[:], AF.Copy,
                                 scale=-0.025 / NTOK, bias=0.025)
            rc8_ps = psS.tile([P, 1], F32, tag="ps_small")
            nc.tensor.matmul(rc8_ps[:], o1x64f_t[:, :P], rc_s[:],
                             start=True, stop=True)
            rc8 = stage.tile([P, 1], F32)
            nc.scalar.activation(rc8[:], rc8_ps[:], AF.Copy)
            pe_o = stage.tile([P, CH], F32, tag="st8", bufs=4)
            nc.vector.tensor_scalar_add(pe_o[:], pe0_s[:], rc8[:])
            nc.sync.dma_start(pe8[:], pe_o[:])

            # own-token conf ([8, CH], row = chunk)
            cert8 = stage.tile([P, CH], F32, tag="st8", bufs=4)
            nc.scalar.activation(cert8[:], pe0_s[:], AF.Abs,
                                 bias=m64n[:P, :])
            temp8 = stage.tile([P, CH], F32, tag="st8", bufs=4)
            sigmoid_of(temp8[:], cert8[:])
            pt8 = stage.tile([P, CH], F32, tag="st8", bufs=4)
            nc.vector.tensor_mul(pt8[:], pe0_s[:], temp8[:])
            th8 = stage.tile([P, CH], F32, tag="st8", bufs=4)
            tanh_neg_of(th8[:], pt8[:], "st8", P)
            conf8 = stage.tile([P, CH], FR)
            nc.scalar.activation(conf8[:], th8[:], AF.Copy,
                                 bias=0.5, scale=0.5)

            # ====================== phase 3 ======================
            for c in range(NCH):
                csl = ds(CH * c, CH)
                pc_f = [outb.tile([128, CH], F32, tag="pc", bufs=5,
                                  name=f"pc_{c}_{i}") for i in range(4)]
                hx = h_r[:, :, csl]
                hy = h_i[:, :, csl]
                pf_r = psH1.tile([128, KT, CH], F32, tag="psH1",
                                 name=f"pfr_{c}")
                pf_i = psH1.tile([128, KT, CH], F32, tag="psH1",
                                 name=f"pfi_{c}")
                for f in range(KT):
                    lin_mm(pf_r[:, f, :], "w3r", "w3in", hx, hy, f, "b3", 0)
                    lin_mm(pf_i[:, f, :], "w3i", "w3r", hx, hy, f, "b3", 1)
                cb_ps = psD.tile([128, CH], F32, tag="psD", name=f"cbp_{c}")
                nc.tensor.matmul(cb_ps[:], bc_t[:, ds(128 * c, 128)],
                                 conf8[:], start=True, stop=True)
                cb = work.tile([128, CH], F32, tag="cb", name=f"cb_{c}")
                nc.scalar.activation(cb[:], cb_ps[:], AF.Copy)
                omc = work.tile([128, CH], F32, tag="omc", name=f"omc_{c}")
                if use_gpsimd:
                    nc.gpsimd.tensor_scalar(omc[:], cb[:], -1.0, 1.0,
                                            op0=ALU.mult, op1=ALU.add)
                else:
                    nc.scalar.activation(omc[:], cb[:], AF.Copy,
                                         bias=1.0, scale=-1.0)
                for f in range(KT):
                    nc.vector.tensor_mul(pc_f[f][:], pf_r[:, f, :], cb[:])
                    nc.vector.tensor_mul(pc_f[2 + f][:], pf_i[:, f, :],
                                         cb[:])
                # cu: overwrite h in place only after BOTH pf f-tiles
                # consumed h (both k slices)
                eng_mul = nc.gpsimd if use_gpsimd else nc.vector
                for f in range(KT):
                    eng_mul.tensor_mul(h_r[:, f, csl], h_r[:, f, csl],
                                       omc[:])
                    eng_mul.tensor_mul(h_i[:, f, csl], h_i[:, f, csl],
                                       omc[:])
                for f in range(KT):
                    nc.sync.dma_start(
                        r3(pc_r)[:, f, csl], pc_f[f][:])
                    nc.sync.dma_start(
                        r3(pc_i)[:, f, csl], pc_f[2 + f][:])
                nc.sync.dma_start(r3(cu_r)[:, :, csl], h_r[:, :, csl])
                nc.sync.dma_start(r3(cu_i)[:, :, csl], h_i[:, :, csl])

    nc.compile()
    return nc


# ----------------------------------------------------------------------
# host-side preparation
# ----------------------------------------------------------------------

def _prep_consts(inp):
    f8 = lambda a: np.asarray(a, np.float64)
    out = {}
    eye = np.eye(D)

    def lin_fold(w1r, w1i, w2r, w2i):
        a, b = f8(w1r), f8(w1i)
        c_, d_ = f8(w2r) - eye, f8(w2i)
        return a @ c_ - b @ d_, a @ d_ + b @ c_

    wdr, wdi = lin_fold(inp["p_w1r"], inp["p_w1i"],
                        inp["p_w2r"], inp["p_w2i"])
    nwdr, nwdi = lin_fold(inp["n_w1r"], inp["n_w1i"],
                          inp["n_w2r"], inp["n_w2i"])
    pairs = [("w1", inp["p_w1r"], inp["p_w1i"]),
             ("wd", wdr, wdi),
             ("nw1", inp["n_w1r"], inp["n_w1i"]),
             ("nwd", nwdr, nwdi),
             ("w3", inp["p_w3r"], inp["p_w3i"])]
    for n, wr, wi in pairs:
        out[n + "r"] = np.ascontiguousarray(wr, dtype=np.float32)
        out[n + "i"] = np.ascontiguousarray(wi, dtype=np.float32)
        out[n + "in"] = np.ascontiguousarray(-f8(wi), dtype=np.float32)

    scale = D ** -0.5
    M = f8(inp["p_patterns"])       # [8, 2D]
    N = f8(inp["n_patterns_bank"])  # [8, 2D]
    out["spmr"] = np.ascontiguousarray((M[:, 0::2] * scale).T, np.float32)
    out["spmi"] = np.ascontiguousarray((M[:, 1::2] * scale).T, np.float32)
    out["spnr"] = np.ascontiguousarray((N[:, 0::2] * scale).T, np.float32)
    out["spni"] = np.ascontiguousarray((N[:, 1::2] * scale).T, np.float32)

    s = 1.0 / (2 * D)
    G11 = (M @ M.T) * s
    G12 = -(M @ N.T) * s
    G21 = G12.T
    G22 = (N @ N.T) * s
    # u_top = G11 @ wm + G12 @ wn ; u_bot = G21 @ wm + G22 @ wn
    out["gtm"] = np.ascontiguousarray(G11.T, np.float32)
    out["gtn"] = np.ascontiguousarray(G12.T, np.float32)
    out["gbm"] = np.ascontiguousarray(G21.T, np.float32)
    out["gbn"] = np.ascontiguousarray(G22.T, np.float32)

    sels = np.zeros((P, P * NCH), np.float32)
    for c in range(NCH):
        sels[:, P * c + c] = 1.0
    out["sels"] = sels
    bcsel = np.zeros((P, 128 * NCH), np.float32)
    for c in range(NCH):
        bcsel[c, 128 * c:128 * (c + 1)] = 1.0
    out["bcsel"] = bcsel
    out["on8"] = np.ones((P, 1), np.float32)
    out["on8f"] = np.ones((P, 1), np.float32)
    out["on64"] = np.ones((N_CORES * P, 1), np.float32)
    out["o1x64"] = np.ones((1, N_CORES * P), np.float32)
    out["o1x64f"] = np.ones((1, N_CORES * P), np.float32)

    bias_vals = {}
    for pre, key in (("", "p"), ("n", "n")):
        b1r, b1i = f8(inp[f"{key}_b1r"]), f8(inp[f"{key}_b1i"])
        b2r, b2i = f8(inp[f"{key}_b2r"]), f8(inp[f"{key}_b2i"])
        w2r, w2i = f8(inp[f"{key}_w2r"]) - eye, f8(inp[f"{key}_w2i"])
        bdr = b1r @ w2r - b1i @ w2i + b2r
        bdi = b1r @ w2i + b1i @ w2r + b2i
        bias_vals[pre + "b1"] = np.stack([b1r, b1i]).astype(np.float32)
        bias_vals[pre + "bd"] = np.stack([bdr, bdi]).astype(np.float32)
    bias_vals["b3"] = np.stack([inp["p_b3r"],
                                inp["p_b3i"]]).astype(np.float32)
    with_bias = any(np.any(v != 0) for v in bias_vals.values())
    if with_bias:
        out.update(bias_vals)
        out["onesrow"] = np.ones((1, CH), np.float32)
    return out, with_bias


_NC_CACHE = {}


def _get_nc(with_bias):
    if with_bias not in _NC_CACHE:
        _NC_CACHE[with_bias] = _build_nc(with_bias)
    return _NC_CACHE[with_bias]


class Runner:
    """Compile once, reuse the jitted sharded callable across calls.

    Vendored from concourse.bass2jax.run_bass_via_pjrt (without buffer
    donation so the callable can be invoked repeatedly for timing)."""

    def __init__(self, nc, n_cores=N_CORES):
        import jax
        from jax.experimental.shard_map import shard_map
        from jax.sharding import Mesh, PartitionSpec
        from concourse import bass2jax, mybir as mb

        bass2jax.install_neuronx_cc_hook()
        assert nc.dbg_addr is None and nc.partition_id_tensor is None or True
        self.nc = nc
        self.n_cores = n_cores
        pid_skip = (nc.partition_id_tensor.name
                    if nc.partition_id_tensor is not None else None)
        in_names, out_names, out_avals, zero_outs = [], [], [], []
        for alloc in nc.m.functions[0].allocations:
            if not isinstance(alloc, mb.MemoryLocationSet):
                continue
            name = alloc.memorylocations[0].name
            if alloc.kind == "ExternalInput":
                if name != pid_skip:
                    in_names.append(name)
            elif alloc.kind == "ExternalOutput":
                shape = tuple(alloc.tensor_shape)
                dtype = mb.dt.np(alloc.dtype)
                out_names.append(name)
                out_avals.append(jax.core.ShapedArray(shape, dtype))
                zero_outs.append(np.zeros(shape, dtype))
        self.in_names = list(in_names)
        self.out_names = out_names
        self.out_avals = out_avals
        self.n_params = len(in_names)
        all_names = self.in_names + out_names
        pid_name = None
        if nc.partition_id_tensor is not None:
            pid_name = nc.partition_id_tensor.name
            all_names.append(pid_name)

        def _body(*args):
            operands = list(args)
            if pid_name is not None:
                operands.append(bass2jax.partition_id_tensor())
            outs = bass2jax._bass_exec_p.bind(
                *operands,
                out_avals=tuple(out_avals),
                in_names=tuple(all_names),
                out_names=tuple(out_names),
                lowering_input_output_aliases=(),
                sim_require_finite=True,
                sim_require_nnan=True,
                nc=nc,
            )
            return tuple(outs)

        devices = jax.devices()[:n_cores]
        mesh = Mesh(np.asarray(devices), ("core",))
        n_in = self.n_params + len(out_names)
        self.sharded = jax.jit(
            shard_map(_body, mesh=mesh,
                      in_specs=(PartitionSpec("core"),) * n_in,
                      out_specs=(PartitionSpec("core"),) * len(out_names),
                      check_rep=False),
            keep_unused=True,
        )
        self.zero_outs = [
            np.zeros((n_cores * z.shape[0], *z.shape[1:]), z.dtype)
            for z in zero_outs
        ]

    def concat_inputs(self, in_maps):
        return [
            np.concatenate([np.asarray(in_maps[c][n])
                            for c in range(self.n_cores)], axis=0)
            for n in self.in_names
        ]

    def run_raw(self, concat_in):
        return self.sharded(*concat_in, *self.zero_outs)

    def __call__(self, in_maps):
        out_arrs = self.run_raw(self.concat_inputs(in_maps))
        return [
            {n: np.asarray(out_arrs[i]).reshape(
                self.n_cores, *self.out_avals[i].shape)[c]
             for i, n in enumerate(self.out_names)}
            for c in range(self.n_cores)
        ]


_RUNNER_CACHE = {}


def get_runner(with_bias):
    if with_bias not in _RUNNER_CACHE:
        _RUNNER_CACHE[with_bias] = Runner(_get_nc(with_bias))
    return _RUNNER_CACHE[with_bias]


def run_device(nc, in_maps):
    from concourse.bass_utils import run_bass_kernel_spmd
    res = run_bass_kernel_spmd(nc, in_maps, core_ids=list(range(N_CORES)))
    return res.results


def make_in_maps(inputs):
    inputs = {k: np.asarray(v) for k, v in inputs.items()}
    consts, with_bias = _prep_consts(inputs)
    in_maps = []
    for c in range(N_CORES):
        m = dict(consts)
        m["x_r"] = np.ascontiguousarray(
            np.asarray(inputs["x_r"][c], np.float32).T)
        m["x_i"] = np.ascontiguousarray(
            np.asarray(inputs["x_i"][c], np.float32).T)
        in_maps.append(m)
    return in_maps, with_bias


def assemble_outputs(results):
    cu_r = np.empty((B, L, D), np.float32)
    cu_i = np.empty((B, L, D), np.float32)
    pc_r = np.empty((B, L, D), np.float32)
    pc_i = np.empty((B, L, D), np.float32)
    pe = np.empty((B, L, 1), np.float32)
    for c in range(N_CORES):
        r = results[c]
        cu_r[c] = r["cu_r"].T
        cu_i[c] = r["cu_i"].T
        pc_r[c] = r["pc_r"].T
        pc_i[c] = r["pc_i"].T
        pe[c] = r["pe8"].reshape(L, 1)
    return (cu_r, cu_i, pc_r, pc_i, pe)


def kernel(**inputs):
    in_maps, with_bias = make_in_maps(inputs)
    runner = get_runner(with_bias)
    results = runner(in_maps)
    return assemble_outputs(results)


# revision 18
# speedup vs baseline: 1.1294x; 1.1294x over previous
"""Trainium2 Bass kernel for nn_DiscretePatternLayer.

Strategy (8 NeuronCores, data-parallel over batch B=8, one batch row per core):

  * Everything on-device is kept feature-major ([D, tokens]) so the matmul
    chain needs no transposes; the host transposes inputs/outputs (numpy).
  * The "paradox" d = h1@W2 + b2 - h1 is folded into host-precomputed
    weights:  d = x @ (W1 @ (W2 - I)) + const-bias  (one complex matmul
    straight from x, accumulated in PSUM).
  * Pattern attention never materializes comp vectors.  pe = mean(
    (next_comp-my_comp)^2 ) is a quadratic form in the two softmax weight
    vectors with a host-precomputed 16x16 Gram matrix of the pattern banks.
  * The two global scalar reductions (mean(pe), routing cost) are handled
    with a single AllGather of the per-token pe vector (16 KB per core);
    every core redundantly computes the global stats (cheap).
  * Token chunks of 512 (one PSUM bank per [128,512] fp32 tile).
"""

import functools

import numpy as np

import concourse.bacc as bacc
import concourse.mybir as mybir
import concourse.tile as tile
from concourse.bass import ds

# ----------------------------------------------------------------------
# All ACT functions this kernel uses ({ln, exp, square, abs, copy}) live
# together in the `natural_log_exp_and_others` table set, but bacc's
# load-insertion pass greedily assigns each function to the FIRST set
# containing it (ln -> natural_log, exp -> exp_and_others), forcing a
# ~2.7us table reload between every ln/exp pair.  Patch the table map so
# functions of the shared set resolve only to it (set ids preserved).
# ----------------------------------------------------------------------
import concourse.hw_specs as _hw_specs  # noqa: E402

_SHARED_SET = "natural_log_exp_and_others"
if not getattr(_hw_specs.get_activation_tables, "_dpl_patched", False):
    _orig_get_tables = _hw_specs.get_activation_tables

    @functools.cache
    def _patched_get_tables(module_arch):
        t = _orig_get_tables(module_arch)
        if _SHARED_SET not in t:
            return t
        shared = t[_SHARED_SET]
        return {name: (fns if name == _SHARED_SET else fns - shared)
                for name, fns in t.items()}

    _patched_get_tables._dpl_patched = True
    _hw_specs.get_activation_tables = _patched_get_tables
    bacc.get_activation_tables = _patched_get_tables

F32 = mybir.dt.float32
FR = mybir.dt.float32r   # "rounded" fp32: full-rate PE (1cyc/row), ~13-bit mantissa
AF = mybir.ActivationFunctionType
ALU = mybir.AluOpType

N_CORES = 8
B, L, D, P = 8, 4096, 256, 8
CH = 512            # tokens per chunk
NCH = L // CH       # chunks per core
KT = D // 128       # k-tiles per feature dim (2)
NTOK = B * L        # global token count

WNAMES = ["w1r", "w1i", "w1in",      # h1 = x @ W1
          "wdr", "wdi", "wdin",      # d  = x @ (W1 @ (W2 - I))
          "nw1r", "nw1i", "nw1in",   # a1 = h @ nW1
          "nwdr", "nwdi", "nwdin",   # da = h @ (nW1 @ (nW2 - I))
          "w3r", "w3i", "w3in"]      # pf = h @ W3


class _Pair:
    """Adapter so two separate [128, CH] tiles index like [128, KT, CH]."""

    def __init__(self, t0, t1):
        self.t = (t0, t1)

    def __getitem__(self, idx):
        _, k, sl = idx
        return self.t[k][:, sl]


def _build_nc(with_bias: bool, single_core: bool = False,
              use_gpsimd: bool = True, use_collective: bool = True):
    ncore = 1 if single_core else N_CORES
    nc = bacc.Bacc("TRN2", target_bir_lowering=False, debug=False,
                   num_devices=ncore)

    dt = lambda name, shape, dty=F32: nc.dram_tensor(
        name, shape, dty, kind="ExternalInput").ap()
    do = lambda name, shape, dty=F32: nc.dram_tensor(
        name, shape, dty, kind="ExternalOutput").ap()

    x_r = dt("x_r", [D, L], FR)
    x_i = dt("x_i", [D, L], FR)
    wpack = dt("wpack", [len(WNAMES), D, D], FR)
    W = {n: wpack[i] for i, n in enumerate(WNAMES)}
    packA = dt("packA", [D, 4 * P], FR)
    pats = {n: packA[:, ds(P * i, P)]
            for i, n in enumerate(["spmr", "spmi", "spnr", "spni"])}
    # packB cols: gram(4x8) | sels(64) | bcsel(1024) | on8(1)
    packB = dt("packB", [P, 32 + P * NCH + 128 * NCH + 1], FR)
    gr = {n: packB[:, ds(P * i, P)]
          for i, n in enumerate(["gtm", "gtn", "gbm", "gbn"])}
    sels = packB[:, ds(32, P * NCH)]
    bcsel = packB[:, ds(32 + P * NCH, 128 * NCH)]
    on8 = packB[:, ds(32 + P * NCH + 128 * NCH, 1)]
    fpack = dt("fpack", [N_CORES * P, 2])   # f32 cols: on64 | on8f
    on64 = fpack[:, ds(0, 1)]
    on8f = fpack[:P, ds(1, 1)]
    o1x64 = dt("o1x64", [1, N_CORES * P], FR)
    o1x64f = dt("o1x64f", [1, N_CORES * P])
    bias = {}
    if with_bias:
        bias = {n: dt(n, [2, D], FR)
                for n in ["b1", "bd", "nb1", "nbd", "b3"]}
        onesrow = dt("onesrow", [1, CH], FR)

    opack = do("opack", [4, D, L], FR)   # cu_r | cu_i | pc_r | pc_i
    cu_r, cu_i, pc_r, pc_i = (opack[i] for i in range(4))
    pe8 = do("pe8", [NCH, CH])

    r3 = lambda ap: ap.rearrange("(kt k) t -> k kt t", k=128)

    with tile.TileContext(nc) as tc:
        with (
            tc.tile_pool(name="consts", bufs=1) as consts,
            tc.tile_pool(name="hbuf", bufs=1) as hbuf,
            tc.tile_pool(name="xin", bufs=2) as xin,
            tc.tile_pool(name="work", bufs=2) as work,
            tc.tile_pool(name="small", bufs=2) as small,
            tc.tile_pool(name="stage", bufs=1) as stage,
            tc.tile_pool(name="outb", bufs=2) as outb,
            tc.tile_pool(name="psD", bufs=2, space="PSUM") as psD,
            tc.tile_pool(name="psH1", bufs=2, space="PSUM") as psH1,
            tc.tile_pool(name="psS", bufs=2, space="PSUM") as psS,
            tc.tile_pool(name="dram", bufs=1, space="DRAM") as dram,
        ):
            # ---------------- constants into SBUF ----------------
            # load the weights needed by the first matmuls first so the PE
            # can start while the rest stream in
            first_w = ["wdr", "wdin", "wdi", "w1r", "w1in", "w1i"]
            wt = {}
            for n in first_w + [n for n in WNAMES if n not in first_w]:
                wt[n] = consts.tile([128, KT, D], FR, name=f"wt_{n}")
                nc.sync.dma_start(wt[n][:], r3(W[n]))
            pt = {}
            for n in pats:
                pt[n] = consts.tile([128, KT, P], FR, name=f"pt_{n}")
                nc.sync.dma_start(pt[n][:], r3(pats[n]))
            gt = {}
            for n in gr:
                gt[n] = consts.tile([P, P], FR, name=f"gt_{n}")
                nc.sync.dma_start(gt[n][:], gr[n][:])
            sel_t = consts.tile([P, P * NCH], FR)
            nc.sync.dma_start(sel_t[:], sels[:])
            bc_t = consts.tile([P, 128 * NCH], FR)
            nc.sync.dma_start(bc_t[:], bcsel[:])
            on8_t = consts.tile([P, 1], FR)
            nc.sync.dma_start(on8_t[:], on8[:])
            on8f_t = consts.tile([P, 1], F32)
            nc.sync.dma_start(on8f_t[:], on8f[:])
            on64_t = consts.tile([N_CORES * P, 1], F32)
            nc.sync.dma_start(on64_t[:], on64[:])
            o1x64_t = consts.tile([1, N_CORES * P], FR)
            nc.sync.dma_start(o1x64_t[:], o1x64[:])
            o1x64f_t = consts.tile([1, N_CORES * P], F32)
            nc.sync.dma_start(o1x64f_t[:], o1x64f[:])
            bias_t = {}
            ones_row = None
            if with_bias:
                for n in bias:
                    bias_t[n] = consts.tile([2, D], FR, name=f"bias_{n}")
                    nc.sync.dma_start(bias_t[n][:], bias[n][:])
                ones_row = consts.tile([1, CH], FR)
                nc.sync.dma_start(ones_row[:], onesrow[:])

            # persistent h (feature-major, full core-local sequence)
            h_r = hbuf.tile([128, KT, L], FR)
            h_i = hbuf.tile([128, KT, L], FR)

            pe0_sb = stage.tile([P, CH], F32)  # accumulated across chunks
            nc.vector.memset(pe0_sb[:], 0.0)

            def lin_mm(psum, wa, wb, rhs_r, rhs_i, f, bias_name, comp):
                """psum[128,CH] = rhs_r @ Wa + rhs_i @ Wb (+bias), out tile f."""
                ms = ds(128 * f, 128)
                nb = with_bias and bias_name is not None
                for j, (w_, rhs) in enumerate([(wa, rhs_r), (wb, rhs_i)]):
                    for k in range(KT):
                        nc.tensor.matmul(
                            psum[:], wt[w_][:, k, ms], rhs[:, k, slice(None)],
                            start=(j == 0 and k == 0),
                            stop=(j == 1 and k == KT - 1 and not nb))
                if nb:
                    nc.tensor.matmul(
                        psum[:], bias_t[bias_name][comp:comp + 1, ms],
                        ones_row[:], start=False, stop=True)

            def paradox_gate(xr_, xi_, wpre, bias_d, bias_1, tag, outs):
                """outs: 2 APs [128, KT, CH] receiving gated h (r, i)."""
                sqr = work.tile([128, KT, CH], F32, tag="sqr", bufs=1,
                                name=f"sqr_{tag}")
                g2 = work.tile([128, KT, CH], F32, tag="g", name=f"g_{tag}")
                for f in range(KT):
                    d_r = psD.tile([128, CH], F32, tag="psD",
                                   name=f"dr_{tag}{f}")
                    d_i = psD.tile([128, CH], F32, tag="psD",
                                   name=f"di_{tag}{f}")
                    lin_mm(d_r, wpre + "wdr", wpre + "wdin",
                           xr_, xi_, f, bias_d, 0)
                    lin_mm(d_i, wpre + "wdi", wpre + "wdr",
                           xr_, xi_, f, bias_d, 1)
                    nc.scalar.activation(sqr[:, f, :], d_r[:], AF.Square)
                    nc.scalar.activation(g2[:, f, :], d_i[:], AF.Square)
                # g = sigmoid(sqrt(m)) via {ln, exp} (single ACT table set),
                # batched over both feature tiles
                eng_add = nc.gpsimd if use_gpsimd else nc.vector
                eng_add.tensor_add(g2[:], g2[:], sqr[:])
                nc.scalar.activation(g2[:], g2[:], AF.Ln)
                nc.scalar.activation(g2[:], g2[:], AF.Exp, scale=0.5)
                nc.scalar.activation(g2[:], g2[:], AF.Exp, scale=-1.0)
                nc.vector.tensor_scalar_add(g2[:], g2[:], 1.0)
                nc.vector.reciprocal(g2[:], g2[:])
                for ci, comp in enumerate(("r", "i")):
                    if comp == "r":
                        wa, wb = wpre + "w1r", wpre + "w1in"
                    else:
                        wa, wb = wpre + "w1i", wpre + "w1r"
                    h1p = psH1.tile([128, KT, CH], F32, tag="psH1",
                                    name=f"h1_{tag}_{comp}")
                    for f in range(KT):
                        lin_mm(h1p[:, f, :], wa, wb, xr_, xi_, f, bias_1, ci)
                    nc.vector.tensor_mul(outs[ci], h1p[:], g2[:])

            def attn_w(t_r, t_i, pre, tag):
                """softmax weights [8, CH] (sbuf) for pattern bank `pre`."""
                sc = psS.tile([P, CH], F32, tag="ps_small", name=f"sc_{tag}")
                tls = {"r": t_r, "i": t_i}
                for j, comp in enumerate(("r", "i")):
                    for k in range(KT):
                        nc.tensor.matmul(
                            sc[:], pt["sp" + pre + comp][:, k, :],
                            tls[comp][:, k, :],
                            start=(j == 0 and k == 0),
                            stop=(j == 1 and k == KT - 1))
                e = small.tile([P, CH], FR, tag="e", name=f"e_{tag}")
                nc.scalar.activation(e[:], sc[:], AF.Exp)
                z = psS.tile([1, CH], F32, tag="ps_small", name=f"z_{tag}")
                nc.tensor.matmul(z[:], on8_t[:], e[:], start=True, stop=True)
                rz = small.tile([1, CH], FR, tag="rz", name=f"rz_{tag}")
                with nc.allow_low_precision(reason="f32r softmax denom"):
                    nc.vector.reciprocal(rz[:], z[:])
                zb = psS.tile([P, CH], F32, tag="ps_small", name=f"zb_{tag}")
                nc.tensor.matmul(zb[:], o1x64_t[:, :P], rz[:],
                                 start=True, stop=True)
                w_ = small.tile([P, CH], FR, tag="w", bufs=4,
                                name=f"w_{tag}")
                nc.vector.tensor_mul(w_[:], e[:], zb[:])
                return w_

            # ====================== phase 1 ======================
            for c in range(NCH):
                csl = ds(CH * c, CH)
                xr = xin.tile([128, KT, CH], FR, tag="xr", name=f"xr_{c}")
                nc.sync.dma_start(xr[:], r3(x_r)[:, :, csl])
                xi = xin.tile([128, KT, CH], FR, tag="xi", name=f"xi_{c}")
                nc.sync.dma_start(xi[:], r3(x_i)[:, :, csl])

                hr_c = h_r[:, :, csl]
                hi_c = h_i[:, :, csl]
                paradox_gate(xr, xi, "", "bd", "b1", f"p{c}",
                             [hr_c, hi_c])

                a_r = work.tile([128, KT, CH], FR, tag="ar", name=f"ar_{c}")
                a_i = work.tile([128, KT, CH], FR, tag="ai", name=f"ai_{c}")
                paradox_gate(hr_c, hi_c, "n", "nbd", "nb1", f"n{c}",
                             [a_r[:], a_i[:]])

                w_my = attn_w(hr_c, hi_c, "m", f"m{c}")
                w_nx = attn_w(a_r[:], a_i[:], "n", f"n{c}")

                u_top = psS.tile([P, CH], F32, tag="ps_small", name=f"ut_{c}")
                nc.tensor.matmul(u_top[:], gt["gtm"][:], w_my[:],
                                 start=True, stop=False)
                nc.tensor.matmul(u_top[:], gt["gtn"][:], w_nx[:],
                                 start=False, stop=True)
                u_bot = psS.tile([P, CH], F32, tag="ps_small", name=f"ub_{c}")
                nc.tensor.matmul(u_bot[:], gt["gbm"][:], w_my[:],
                                 start=True, stop=False)
                nc.tensor.matmul(u_bot[:], gt["gbn"][:], w_nx[:],
                                 start=False, stop=True)
                pm = small.tile([P, CH], FR, tag="pm", name=f"pm_{c}")
                nc.vector.tensor_mul(pm[:], w_my[:], u_top[:])
                pn_ = small.tile([P, CH], FR, tag="pn", name=f"pn_{c}")
                nc.vector.tensor_mul(pn_[:], w_nx[:], u_bot[:])
                ssl = ds(P * c, P)
                pe_c = psS.tile([P, CH], F32, tag="ps_small",
                                name=f"pec_{c}")
                nc.tensor.matmul(pe_c[:], sel_t[:, ssl], pm[:],
                                 start=True, stop=False)
                nc.tensor.matmul(pe_c[:], sel_t[:, ssl], pn_[:],
                                 start=False, stop=True)
                nc.vector.tensor_add(pe0_sb[:], pe0_sb[:], pe_c[:])

            # ====================== phase 2 ======================
            pe0_s = pe0_sb
            s8 = stage.tile([P, 1], F32)
            nc.vector.reduce_sum(s8[:], pe0_s[:], axis=mybir.AxisListType.X)

            bounce_in = dram.tile([P, CH], F32)
            nc.sync.dma_start(bounce_in[:], pe0_s[:])
            bounce_out = dram.tile([N_CORES * P, CH], F32)
            if single_core or not use_collective:
                # stand-in with similar traffic for TimelineSim analysis
                for cc in range(N_CORES):
                    nc.sync.dma_start(bounce_out[ds(P * cc, P), :],
                                      bounce_in[:])
            else:
                nc.gpsimd.collective_compute(
                    "AllGather", ALU.bypass,
                    replica_groups=[list(range(N_CORES))],
                    ins=[bounce_in.opt()], outs=[bounce_out.opt()],
                )
            pall = stage.tile([N_CORES * P, CH], F32)
            nc.sync.dma_start(pall[:], bounce_out[:])

            tot_ps = psS.tile([1, 1], F32, tag="ps_small")
            nc.tensor.matmul(tot_ps[:], on8f_t[:], s8[:], start=True,
                             stop=True)
            t_s = stage.tile([1, 1], F32)
            nc.scalar.activation(t_s[:], tot_ps[:], AF.Copy, scale=1.0 / NTOK)
            m64_ps = psS.tile([N_CORES * P, 1], F32, tag="ps_small")
            nc.tensor.matmul(m64_ps[:], o1x64f_t[:], t_s[:],
                             start=True, stop=True)
            m64n = stage.tile([N_CORES * P, 1], F32)
            nc.scalar.activation(m64n[:], m64_ps[:], AF.Copy, scale=-1.0)

            def sigmoid_of(dst, src):
                # dst = 1 / (1 + exp(-src))
                nc.scalar.activation(dst, src, AF.Exp, scale=-1.0)
                nc.vector.tensor_scalar_add(dst, dst, 1.0)
                nc.vector.reciprocal(dst, dst)

            def tanh_neg_of(dst, src, pool_tag, nparts):
                # dst = tanh(-src) = (q - 1) / (q + 1), q = exp(-2 src)
                q = stage.tile([nparts, CH], F32, tag=pool_tag,
                               bufs=3 if pool_tag == "st64" else 4,
                               name=f"q_{pool_tag}")
                nc.scalar.activation(q[:], src, AF.Exp, scale=-2.0)
                den = stage.tile([nparts, CH], F32, tag=pool_tag,
                                 bufs=3 if pool_tag == "st64" else 4,
                                 name=f"den_{pool_tag}")
                nc.vector.tensor_scalar_add(den[:], q[:], 1.0)
                nc.vector.reciprocal(den[:], den[:])
                nc.vector.tensor_scalar_add(q[:], q[:], -1.0)
                nc.vector.tensor_mul(dst, q[:], den[:])

            cert = stage.tile([N_CORES * P, CH], F32, tag="st64", bufs=3)
            nc.scalar.activation(cert[:], pall[:], AF.Abs, bias=m64n[:])
            temp = stage.tile([N_CORES * P, CH], F32, tag="st64", bufs=3)
            sigmoid_of(temp[:], cert[:])
            pt64 = stage.tile([N_CORES * P, CH], F32, tag="st64", bufs=3)
            nc.vector.tensor_mul(pt64[:], pall[:], temp[:])
            th64 = stage.tile([N_CORES * P, CH], F32, tag="st64", bufs=3)
            tanh_neg_of(th64[:], pt64[:], "st64", N_CORES * P)
            sq64 = stage.tile([N_CORES * P, CH], F32, tag="st64", bufs=3)
            ss64 = stage.tile([N_CORES * P, 1], F32)
            nc.scalar.activation(sq64[:], th64[:], AF.Square,
                                 accum_out=ss64[:])
            tot2_ps = psS.tile([1, 1], F32, tag="ps_small")
            nc.tensor.matmul(tot2_ps[:], on64_t[:], ss64[:],
                             start=True, stop=True)
            rc_s = stage.tile([1, 1], F32)
            # rc = 0.1 * mean(conf*(1-conf)) = 0.025*(1 - mean(th^2))
            nc.scalar.activation(rc_s[:], tot2_ps[:], AF.Copy,
                                 scale=-0.025 / NTOK, bias=0.025)
            rc8_ps = psS.tile([P, 1], F32, tag="ps_small")
            nc.tensor.matmul(rc8_ps[:], o1x64f_t[:, :P], rc_s[:],
                             start=True, stop=True)
            rc8 = stage.tile([P, 1], F32)
            nc.scalar.activation(rc8[:], rc8_ps[:], AF.Copy)
            pe_o = stage.tile([P, CH], F32, tag="st8", bufs=4)
            nc.vector.tensor_scalar_add(pe_o[:], pe0_s[:], rc8[:])
            nc.sync.dma_start(pe8[:], pe_o[:])

            # own-token conf ([8, CH], row = chunk)
            cert8 = stage.tile([P, CH], F32, tag="st8", bufs=4)
            nc.scalar.activation(cert8[:], pe0_s[:], AF.Abs,
                                 bias=m64n[:P, :])
            temp8 = stage.tile([P, CH], F32, tag="st8", bufs=4)
            sigmoid_of(temp8[:], cert8[:])
            pt8 = stage.tile([P, CH], F32, tag="st8", bufs=4)
            nc.vector.tensor_mul(pt8[:], pe0_s[:], temp8[:])
            th8 = stage.tile([P, CH], F32, tag="st8", bufs=4)
            tanh_neg_of(th8[:], pt8[:], "st8", P)
            conf8 = stage.tile([P, CH], FR)
            nc.scalar.activation(conf8[:], th8[:], AF.Copy,
                                 bias=0.5, scale=0.5)

            # ====================== phase 3 ======================
            for c in range(NCH):
                csl = ds(CH * c, CH)
                pc_f = [outb.tile([128, CH], FR, tag="pc", bufs=5,
                                  name=f"pc_{c}_{i}") for i in range(4)]
                hx = h_r[:, :, csl]
                hy = h_i[:, :, csl]
                pf_r = psH1.tile([128, KT, CH], F32, tag="psH1",
                                 name=f"pfr_{c}")
                pf_i = psH1.tile([128, KT, CH], F32, tag="psH1",
                                 name=f"pfi_{c}")
                for f in range(KT):
                    lin_mm(pf_r[:, f, :], "w3r", "w3in", hx, hy, f, "b3", 0)
                    lin_mm(pf_i[:, f, :], "w3i", "w3r", hx, hy, f, "b3", 1)
                cb_ps = psD.tile([128, CH], F32, tag="psD", name=f"cbp_{c}")
                nc.tensor.matmul(cb_ps[:], bc_t[:, ds(128 * c, 128)],
                                 conf8[:], start=True, stop=True)
                cb = work.tile([128, CH], F32, tag="cb", name=f"cb_{c}")
                nc.scalar.activation(cb[:], cb_ps[:], AF.Copy)
                omc = work.tile([128, CH], F32, tag="omc", name=f"omc_{c}")
                if use_gpsimd:
                    nc.gpsimd.tensor_scalar(omc[:], cb[:], -1.0, 1.0,
                                            op0=ALU.mult, op1=ALU.add)
                else:
                    nc.scalar.activation(omc[:], cb[:], AF.Copy,
                                         bias=1.0, scale=-1.0)
                for f in range(KT):
                    nc.vector.tensor_mul(pc_f[f][:], pf_r[:, f, :], cb[:])
                    nc.vector.tensor_mul(pc_f[2 + f][:], pf_i[:, f, :],
                                         cb[:])
                # cu: overwrite h in place only after BOTH pf f-tiles
                # consumed h (both k slices)
                eng_mul = nc.gpsimd if use_gpsimd else nc.vector
                for f in range(KT):
                    eng_mul.tensor_mul(h_r[:, f, csl], h_r[:, f, csl],
                                       omc[:])
                    eng_mul.tensor_mul(h_i[:, f, csl], h_i[:, f, csl],
                                       omc[:])
                for f in range(KT):
                    nc.sync.dma_start(
                        r3(pc_r)[:, f, csl], pc_f[f][:])
                    nc.sync.dma_start(
                        r3(pc_i)[:, f, csl], pc_f[2 + f][:])
                nc.sync.dma_start(r3(cu_r)[:, :, csl], h_r[:, :, csl])
                nc.sync.dma_start(r3(cu_i)[:, :, csl], h_i[:, :, csl])

    nc.compile()
    return nc


# ----------------------------------------------------------------------
# host-side preparation
# ----------------------------------------------------------------------

def _prep_consts(inp):
    f8 = lambda a: np.asarray(a, np.float64)
    out = {}
    eye = np.eye(D)

    def lin_fold(w1r, w1i, w2r, w2i):
        a, b = f8(w1r), f8(w1i)
        c_, d_ = f8(w2r) - eye, f8(w2i)
        return a @ c_ - b @ d_, a @ d_ + b @ c_

    wdr, wdi = lin_fold(inp["p_w1r"], inp["p_w1i"],
                        inp["p_w2r"], inp["p_w2i"])
    nwdr, nwdi = lin_fold(inp["n_w1r"], inp["n_w1i"],
                          inp["n_w2r"], inp["n_w2i"])
    pairs = [("w1", inp["p_w1r"], inp["p_w1i"]),
             ("wd", wdr, wdi),
             ("nw1", inp["n_w1r"], inp["n_w1i"]),
             ("nwd", nwdr, nwdi),
             ("w3", inp["p_w3r"], inp["p_w3i"])]
    wvals = {}
    for n, wr, wi in pairs:
        wvals[n + "r"] = wr
        wvals[n + "i"] = wi
        wvals[n + "in"] = -f8(wi)
    out["wpack"] = np.ascontiguousarray(
        np.stack([wvals[n] for n in WNAMES]), dtype=np.float32)

    scale = D ** -0.5
    M = f8(inp["p_patterns"])       # [8, 2D]
    N = f8(inp["n_patterns_bank"])  # [8, 2D]
    out["packA"] = np.ascontiguousarray(np.concatenate(
        [(M[:, 0::2] * scale).T, (M[:, 1::2] * scale).T,
         (N[:, 0::2] * scale).T, (N[:, 1::2] * scale).T],
        axis=1), np.float32)

    s = 1.0 / (2 * D)
    G11 = (M @ M.T) * s
    G12 = -(M @ N.T) * s
    G21 = G12.T
    G22 = (N @ N.T) * s
    # u_top = G11 @ wm + G12 @ wn ; u_bot = G21 @ wm + G22 @ wn
    sels = np.zeros((P, P * NCH))
    for c in range(NCH):
        sels[:, P * c + c] = 1.0
    bcsel = np.zeros((P, 128 * NCH))
    for c in range(NCH):
        bcsel[c, 128 * c:128 * (c + 1)] = 1.0
    out["packB"] = np.ascontiguousarray(np.concatenate(
        [G11.T, G12.T, G21.T, G22.T, sels, bcsel, np.ones((P, 1))],
        axis=1), np.float32)
    fpack = np.ones((N_CORES * P, 2))
    out["fpack"] = np.ascontiguousarray(fpack, np.float32)
    out["o1x64"] = np.ones((1, N_CORES * P), np.float32)
    out["o1x64f"] = np.ones((1, N_CORES * P), np.float32)

    bias_vals = {}
    for pre, key in (("", "p"), ("n", "n")):
        b1r, b1i = f8(inp[f"{key}_b1r"]), f8(inp[f"{key}_b1i"])
        b2r, b2i = f8(inp[f"{key}_b2r"]), f8(inp[f"{key}_b2i"])
        w2r, w2i = f8(inp[f"{key}_w2r"]) - eye, f8(inp[f"{key}_w2i"])
        bdr = b1r @ w2r - b1i @ w2i + b2r
        bdi = b1r @ w2i + b1i @ w2r + b2i
        bias_vals[pre + "b1"] = np.stack([b1r, b1i]).astype(np.float32)
        bias_vals[pre + "bd"] = np.stack([bdr, bdi]).astype(np.float32)
    bias_vals["b3"] = np.stack([inp["p_b3r"],
                                inp["p_b3i"]]).astype(np.float32)
    with_bias = any(np.any(v != 0) for v in bias_vals.values())
    if with_bias:
        out.update(bias_vals)
        out["onesrow"] = np.ones((1, CH), np.float32)
    return out, with_bias


_NC_CACHE = {}


def _get_nc(with_bias):
    if with_bias not in _NC_CACHE:
        _NC_CACHE[with_bias] = _build_nc(with_bias)
    return _NC_CACHE[with_bias]


class Runner:
    """Compile once, reuse the jitted sharded callable across calls.

    Vendored from concourse.bass2jax.run_bass_via_pjrt (without buffer
    donation so the callable can be invoked repeatedly for timing)."""

    def __init__(self, nc, n_cores=N_CORES):
        import jax
        from jax.experimental.shard_map import shard_map
        from jax.sharding import Mesh, PartitionSpec
        from concourse import bass2jax, mybir as mb

        bass2jax.install_neuronx_cc_hook()
        assert nc.dbg_addr is None and nc.partition_id_tensor is None or True
        self.nc = nc
        self.n_cores = n_cores
        pid_skip = (nc.partition_id_tensor.name
                    if nc.partition_id_tensor is not None else None)
        in_names, out_names, out_avals, zero_outs = [], [], [], []
        for alloc in nc.m.functions[0].allocations:
            if not isinstance(alloc, mb.MemoryLocationSet):
                continue
            name = alloc.memorylocations[0].name
            if alloc.kind == "ExternalInput":
                if name != pid_skip:
                    in_names.append(name)
            elif alloc.kind == "ExternalOutput":
                shape = tuple(alloc.tensor_shape)
                dtype = mb.dt.np(alloc.dtype)
                out_names.append(name)
                out_avals.append(jax.core.ShapedArray(shape, dtype))
                zero_outs.append(np.zeros(shape, dtype))
        self.in_names = list(in_names)
        self.out_names = out_names
        self.out_avals = out_avals
        self.n_params = len(in_names)
        all_names = self.in_names + out_names
        pid_name = None
        if nc.partition_id_tensor is not None:
            pid_name = nc.partition_id_tensor.name
            all_names.append(pid_name)

        def _body(*args):
            operands = list(args)
            if pid_name is not None:
                operands.append(bass2jax.partition_id_tensor())
            outs = bass2jax._bass_exec_p.bind(
                *operands,
                out_avals=tuple(out_avals),
                in_names=tuple(all_names),
                out_names=tuple(out_names),
                lowering_input_output_aliases=(),
                sim_require_finite=True,
                sim_require_nnan=True,
                nc=nc,
            )
            return tuple(outs)

        devices = jax.devices()[:n_cores]
        mesh = Mesh(np.asarray(devices), ("core",))
        n_in = self.n_params + len(out_names)
        self.sharded = jax.jit(
            shard_map(_body, mesh=mesh,
                      in_specs=(PartitionSpec("core"),) * n_in,
                      out_specs=(PartitionSpec("core"),) * len(out_names),
                      check_rep=False),
            keep_unused=True,
        )
        self.zero_outs = [
            np.zeros((n_cores * z.shape[0], *z.shape[1:]), z.dtype)
            for z in zero_outs
        ]

    def concat_inputs(self, in_maps):
        return [
            np.concatenate([np.asarray(in_maps[c][n])
                            for c in range(self.n_cores)], axis=0)
            for n in self.in_names
        ]

    def run_raw(self, concat_in):
        return self.sharded(*concat_in, *self.zero_outs)

    def __call__(self, in_maps):
        out_arrs = self.run_raw(self.concat_inputs(in_maps))
        return [
            {n: np.asarray(out_arrs[i]).reshape(
                self.n_cores, *self.out_avals[i].shape)[c]
             for i, n in enumerate(self.out_names)}
            for c in range(self.n_cores)
        ]


_RUNNER_CACHE = {}


def get_runner(with_bias):
    if with_bias not in _RUNNER_CACHE:
        _RUNNER_CACHE[with_bias] = Runner(_get_nc(with_bias))
    return _RUNNER_CACHE[with_bias]


def run_device(nc, in_maps):
    from concourse.bass_utils import run_bass_kernel_spmd
    res = run_bass_kernel_spmd(nc, in_maps, core_ids=list(range(N_CORES)))
    return res.results


def make_in_maps(inputs):
    inputs = {k: np.asarray(v) for k, v in inputs.items()}
    consts, with_bias = _prep_consts(inputs)
    in_maps = []
    for c in range(N_CORES):
        m = dict(consts)
        m["x_r"] = np.ascontiguousarray(
            np.asarray(inputs["x_r"][c], np.float32).T)
        m["x_i"] = np.ascontiguousarray(
            np.asarray(inputs["x_i"][c], np.float32).T)
        in_maps.append(m)
    return in_maps, with_bias


def assemble_outputs(results):
    cu_r = np.empty((B, L, D), np.float32)
    cu_i = np.empty((B, L, D), np.float32)
    pc_r = np.empty((B, L, D), np.float32)
    pc_i = np.empty((B, L, D), np.float32)
    pe = np.empty((B, L, 1), np.float32)
    for c in range(N_CORES):
        r = results[c]
        op = r["opack"]
        cu_r[c] = op[0].T
        cu_i[c] = op[1].T
        pc_r[c] = op[2].T
        pc_i[c] = op[3].T
        pe[c] = r["pe8"].reshape(L, 1)
    return (cu_r, cu_i, pc_r, pc_i, pe)


def kernel(**inputs):
    in_maps, with_bias = make_in_maps(inputs)
    runner = get_runner(with_bias)
    results = runner(in_maps)
    return assemble_outputs(results)
